# revision 5
# baseline (speedup 1.0000x reference)
"""Trainium2 Bass kernel for batched dense attention (v4).

Reference (per batch b):
    q = query @ Wq + bq ; k = key @ Wk + bk ; v = value @ Wv + bv
    out = softmax(BETA * q k^T) v

Sharding: 8 cores = (batch b, seq half h). Core (b,h) computes out rows
[b, h*1024:(h+1)*1024, :]. K-side work is duplicated across the two cores
of a batch (no collectives).

Design:
  - All matmul inputs host-cast bf16 and HOST PRE-ARRANGED in the exact
    SBUF chunk-concatenated layout ([128, chunk, cols]) so every DMA is a
    plain contiguous 2D copy at full line efficiency (v3's 3D chunked APs
    ran at ~100GB/s and stalled PE 19us; 2D runs at ~400GB/s).
  - query/key arrive transposed ([D, rows] chunked): zero PE transposes.
  - Projections: q-block0 runs c-outer (consume chunks as DMA lands);
    later blocks run g-outer so each PSUM accumulator finishes early and
    its bias-add (DVE tensor_scalar) overlaps the next chain -- avoids a
    ~5us DVE drain at the PSUM pool boundary before the main loop.
  - Phase A: sT[k,q] tiles -> exp on ScalarE -> pT (bf16); P row sums
    accumulate on PE (lhsT=pT slice, rhs=ones). The rowsum matmul for tile
    kt-2 is interleaved at positions g>=4 of tile kt's score chain so its
    LDWEIGHTS hides under the 512-cycle score streams.
  - Phase B: out2T[vd', q] = (value^T P)^T using natural-layout value tiles
    as lhsT (contraction over k) -- no transposes.
  - Phase C: out[q,:] = (out2T^T @ Wv) * (1/rowsum) + bv, normalization and
    bias fused in one scalar_tensor_tensor on the PSUM->SBUF copy.

Per-core PE budget @2.4GHz: qproj 65k + kproj 131k + scores 131k +
phaseB 131k + phaseC 65k ~= 523k cycles ~= 218us.
"""
import ml_dtypes
import numpy as np

import concourse.bass as bass
import concourse.bacc as bacc
import concourse.tile as tile
from concourse import mybir
from concourse.bass_utils import run_bass_kernel_spmd

B, S, D = 4, 2048, 1024
KD = 1024
VD = 1024
BETA = 1.0 / float(np.sqrt(D))
N_CORES = 8
QS = S // 2          # per-core query rows (1024)

F32 = mybir.dt.float32
BF16 = mybir.dt.bfloat16

C_D = D // 128       # 8 contraction chunks over D
G_KD = KD // 128     # 8 kd chunks
KT = S // 128        # 16 key tiles
QBLK = 512
NQB = QS // QBLK     # 2 q blocks
NQS = QBLK // 128    # 4 q slices per block
NKB = S // 512       # 4 key col-blocks


def build_kernel():
    nc = bacc.Bacc("TRN2", target_bir_lowering=False, debug=False,
                   num_devices=N_CORES)

    # host-prearranged [128, ...] layouts (see make_in_maps)
    qTh = nc.dram_tensor("qTh", [128, NQB * C_D * 512], BF16,
                         kind="ExternalInput").ap()
    kTh = nc.dram_tensor("kTh", [128, NKB * C_D * 512], BF16,
                         kind="ExternalInput").ap()
    v16h = nc.dram_tensor("v16h", [128, 2 * KT * 512], BF16,
                          kind="ExternalInput").ap()
    Wqh = nc.dram_tensor("Wqh", [128, C_D * KD], BF16,
                         kind="ExternalInput").ap()
    Wkh = nc.dram_tensor("Wkh", [128, C_D * KD], BF16,
                         kind="ExternalInput").ap()
    Wvh = nc.dram_tensor("Wvh", [128, C_D * VD], BF16,
                         kind="ExternalInput").ap()
    bqk = nc.dram_tensor("bqk", [128, 16], F32, kind="ExternalInput").ap()
    bv = nc.dram_tensor("bv", [VD], F32, kind="ExternalInput").ap()
    out = nc.dram_tensor("out", [QS, VD], F32, kind="ExternalOutput").ap()

    with tile.TileContext(nc) as tc:
        _body(tc, qTh, kTh, v16h, Wqh, Wkh, Wvh, bqk, bv, out)
    nc.compile()
    return nc


def _body(tc, qTh, kTh, v16h, Wqh, Wkh, Wvh, bqk, bv, out):
    nc = tc.nc
    Exp = mybir.ActivationFunctionType.Exp
    mult = mybir.AluOpType.mult
    add = mybir.AluOpType.add

    # ---- persistent constants ------------------------------------------
    const_pool = tc.alloc_tile_pool(name="const", bufs=1)
    constf = const_pool.tile([128, 2210], F32, name="constf")
    bqT = constf[:, 0:8]
    bkT = constf[:, 8:16]
    bvb = constf[:, 16:16 + VD]
    ones_f = constf[:, 1040:1042]
    rrec_all = constf[:, 1042:1058]
    onesrow_f = constf[0:1, 1058:1058 + 128]
    bv_stage = constf[0:1, 1186:1186 + VD]
    onesb = const_pool.tile([128, 2], BF16, name="onesb")

    nc.vector.memset(ones_f, 1.0)
    nc.vector.memset(onesrow_f, 1.0)
    nc.vector.tensor_copy(onesb[:], ones_f)

    # ---- persistent activations ----------------------------------------
    big_pool = tc.alloc_tile_pool(name="big", bufs=1)
    qTr = big_pool.tile([128, G_KD * QS], BF16, name="qTr")       # 16KB/p
    kTr = big_pool.tile([128, G_KD * S], BF16, name="kTr")        # 32KB/p
    Wv_sb = big_pool.tile([128, C_D * VD], BF16, name="Wv_sb")    # 16KB/p
    pT = big_pool.tile([128, KT * QBLK], BF16, name="pT")         # 16KB/p
    o2T = big_pool.tile([128, C_D * QBLK], BF16, name="o2T")      # 8KB/p
    ost_all = big_pool.tile([128, 2 * 1024], F32, name="ost_all")  # 8KB/p
    ostage = [ost_all[:, i * 1024:(i + 1) * 1024] for i in range(2)]

    # ---- projection-phase transients -----------------------------------
    proj_pool = tc.alloc_tile_pool(name="proj", bufs=1)
    Wq_sb = proj_pool.tile([128, C_D * KD], BF16, name="Wq_sb")   # 16KB/p
    Wk_sb = proj_pool.tile([128, C_D * KD], BF16, name="Wk_sb")   # 16KB/p
    qblk = [proj_pool.tile([128, C_D * 512], BF16, name=f"qb{i}")
            for i in range(NQB)]                                  # 2x8KB/p
    kblk = [proj_pool.tile([128, C_D * 512], BF16, name=f"kb{i}")
            for i in range(NKB)]                                  # 4x8KB/p

    # scalar queue: q block 0 first (first matmul gate), then biases, Wk,
    # kT blocks 0-1
    nc.scalar.dma_start(out=qblk[0][:], in_=qTh[:, 0:C_D * 512])
    nc.scalar.dma_start(out=constf[:, 0:16], in_=bqk[:, :])
    nc.scalar.dma_start(out=bv_stage, in_=bv[:])
    nc.scalar.dma_start(out=Wk_sb[:], in_=Wkh[:, :])
    nc.scalar.dma_start(out=kblk[0][:], in_=kTh[:, 0:C_D * 512])
    nc.scalar.dma_start(out=kblk[1][:], in_=kTh[:, C_D * 512:2 * C_D * 512])
    # sync queue: Wq halves, q block 1, kT blocks 2-3
    nc.sync.dma_start(out=Wq_sb[:, 0:4 * KD], in_=Wqh[:, 0:4 * KD])
    nc.sync.dma_start(out=Wq_sb[:, 4 * KD:8 * KD], in_=Wqh[:, 4 * KD:8 * KD])
    nc.sync.dma_start(out=qblk[1][:], in_=qTh[:, C_D * 512:2 * C_D * 512])
    nc.sync.dma_start(out=kblk[2][:], in_=kTh[:, 2 * C_D * 512:3 * C_D * 512])
    nc.sync.dma_start(out=kblk[3][:], in_=kTh[:, 3 * C_D * 512:4 * C_D * 512])
    # gpsimd queue: Wv (needed at phase C), value streamed in main loop
    nc.gpsimd.dma_start(out=Wv_sb[:], in_=Wvh[:, :])

    psPro = tc.alloc_tile_pool(name="psPro", bufs=1, space="PSUM")

    def proj_block(W_sb, src_blk, dstT, dst_len, blk, bias, c_outer, nm):
        pps = [psPro.tile([128, 512], F32, name=f"{nm}{blk}_{g}", tag="pp",
                          bufs=8) for g in range(G_KD)]

        def ts(g):
            nc.vector.tensor_scalar(
                out=dstT[:, g * dst_len + blk * 512:
                         g * dst_len + (blk + 1) * 512],
                in0=pps[g][:], scalar1=bias[:, g:g + 1], scalar2=None,
                op0=add)

        if c_outer:
            for c in range(C_D):
                for g in range(G_KD):
                    nc.tensor.matmul(
                        pps[g][:],
                        W_sb[:, c * KD + g * 128:c * KD + (g + 1) * 128],
                        src_blk[:, c * 512:(c + 1) * 512],
                        start=(c == 0), stop=(c == C_D - 1))
            for g in range(G_KD):
                ts(g)
        else:
            for g in range(G_KD):
                for c in range(C_D):
                    nc.tensor.matmul(
                        pps[g][:],
                        W_sb[:, c * KD + g * 128:c * KD + (g + 1) * 128],
                        src_blk[:, c * 512:(c + 1) * 512],
                        start=(c == 0), stop=(c == C_D - 1))
                ts(g)

    # ---- q projection: qTr[kd, q] = (Wq^T qT) + bq ----------------------
    proj_block(Wq_sb, qblk[0], qTr, QS, 0, bqT, True, "qp")

    # bv broadcast to all partitions via K=1 fp32 matmul (off the
    # pool-boundary critical path)
    for n in range(VD // 512):
        bc_ps = psPro.tile([128, 512], F32, name="bc_ps", tag="pp", bufs=8)
        nc.tensor.matmul(bc_ps[:], onesrow_f,
                         bv_stage[:, n * 512:(n + 1) * 512],
                         start=True, stop=True)
        nc.vector.tensor_copy(bvb[:, n * 512:(n + 1) * 512], bc_ps[:])

    proj_block(Wq_sb, qblk[1], qTr, QS, 1, bqT, False, "qp")

    # ---- k projection: kTr[kd, k] = (Wk^T kT) + bk ----------------------
    for blk in range(NKB):
        proj_block(Wk_sb, kblk[blk], kTr, S, blk, bkT, False, "kp")

    psPro.release()
    proj_pool.release()

    # ===== main attention loop ==========================================
    # PSUM: sT(2) + rs(1) + acc(4) = 7 banks.
    psM = tc.alloc_tile_pool(name="psM", bufs=1, space="PSUM")
    rs_ps = psM.tile([128, 2 * NQS], F32, name="rs_ps", tag="rs")

    for qb in range(NQB):
        q0 = qb * QBLK

        def rs_mm(kt, qs):
            nc.tensor.matmul(
                rs_ps[:, 2 * qs:2 * qs + 2],
                pT[:, kt * QBLK + qs * 128:kt * QBLK + (qs + 1) * 128],
                onesb[:],
                start=(kt == 0 and qs == 0),
                stop=(kt == KT - 1 and qs == NQS - 1),
                skip_group_check=True)

        # ---- phase A: sT = kTr^T qTr -> exp -> pT ; rowsums on PE ------
        for kt in range(KT):
            sT = psM.tile([128, QBLK], F32, name=f"sT{qb}_{kt}", tag="sT",
                          bufs=2)
            for g in range(G_KD):
                nc.tensor.matmul(
                    sT[:],
                    kTr[:, g * S + kt * 128:g * S + (kt + 1) * 128],
                    qTr[:, g * QS + q0:g * QS + q0 + QBLK],
                    start=(g == 0), stop=(g == G_KD - 1))
                if kt >= 2 and g >= 4:
                    rs_mm(kt - 2, g - 4)
            nc.scalar.activation(pT[:, kt * QBLK:(kt + 1) * QBLK], sT[:],
                                 Exp, scale=float(BETA))
        for kt in (KT - 2, KT - 1):
            for qs in range(NQS):
                rs_mm(kt, qs)
        rrec = rrec_all[:, qb * 2 * NQS:(qb + 1) * 2 * NQS]
        nc.vector.reciprocal(rrec, rs_ps[:])

        # ---- phase B: o2T[vd', q] = (value^T P)^T via lhsT=value tiles --
        for p in range(2):
            accs = [psM.tile([128, QBLK], F32, name=f"o2{qb}_{p}_{u}",
                             tag="acc", bufs=4) for u in range(4)]
            for kt4 in range(KT // 4):
                vt = big_pool.tile([128, 4 * 512], BF16,
                                   name=f"vt{qb}_{p}_{kt4}", tag="vring",
                                   bufs=3)
                nc.gpsimd.dma_start(
                    out=vt[:],
                    in_=v16h[:, p * (KT * 512) + kt4 * 2048:
                             p * (KT * 512) + (kt4 + 1) * 2048])
                for j in range(4):
                    kt = 4 * kt4 + j
                    for u in range(4):
                        nc.tensor.matmul(
                            accs[u][:],
                            vt[:, j * 512 + u * 128:j * 512 + (u + 1) * 128],
                            pT[:, kt * QBLK:(kt + 1) * QBLK],
                            start=(kt == 0), stop=(kt == KT - 1))
            for u in range(4):
                nc.vector.tensor_copy(
                    o2T[:, (4 * p + u) * QBLK:(4 * p + u + 1) * QBLK],
                    accs[u][:])

        # ---- phase C: out = (o2T^T Wv) * rrec + bv ----------------------
        for qs in range(NQS):
            ost = ostage[qs % 2]
            for col in range(2):
                op = psM.tile([128, 512], F32, name=f"op{qb}_{qs}_{col}",
                              tag="acc", bufs=4)
                for cp in range(C_D):
                    nc.tensor.matmul(
                        op[:],
                        o2T[:, cp * QBLK + qs * 128:cp * QBLK + (qs + 1) * 128],
                        Wv_sb[:, cp * VD + col * 512:cp * VD + (col + 1) * 512],
                        start=(cp == 0), stop=(cp == C_D - 1))
                nc.vector.scalar_tensor_tensor(
                    out=ost[:, col * 512:(col + 1) * 512], in0=op[:],
                    scalar=rrec[:, 2 * qs:2 * qs + 1],
                    in1=bvb[:, col * 512:(col + 1) * 512], op0=mult, op1=add)
            nc.sync.dma_start(
                out=out[q0 + qs * 128:q0 + (qs + 1) * 128, :], in_=ost[:])

    psM.release()
    big_pool.release()
    const_pool.release()


_NC_CACHE = {}


def _get_nc():
    if "nc" not in _NC_CACHE:
        _NC_CACHE["nc"] = build_kernel()
    return _NC_CACHE["nc"]


def kernel(query, key, value, Wq, bq, Wk, bk, Wv, bv):
    query = np.asarray(query, dtype=np.float32)
    key = np.asarray(key, dtype=np.float32)
    value = np.asarray(value, dtype=np.float32)
    Wq = np.asarray(Wq, dtype=np.float32)
    Wk = np.asarray(Wk, dtype=np.float32)
    Wv = np.asarray(Wv, dtype=np.float32)
    bq = np.asarray(bq, dtype=np.float32)
    bk = np.asarray(bk, dtype=np.float32)
    bv = np.ascontiguousarray(np.asarray(bv, dtype=np.float32))

    nc = _get_nc()
    in_maps = make_in_maps(query, key, value, Wq, bq, Wk, bk, Wv, bv)
    res = run_bass_kernel_spmd(nc, in_maps, list(range(N_CORES)))
    outp = np.empty((B, S, VD), dtype=np.float32)
    for core in range(N_CORES):
        b, h = divmod(core, 2)
        outp[b, h * QS:(h + 1) * QS, :] = res.results[core]["out"]
    return outp


def _arrange_w(W):
    """[D, N] f32 -> bf16 [128, C_D*N] with chunk-major columns."""
    Dn, N = W.shape
    return np.ascontiguousarray(
        W.astype(ml_dtypes.bfloat16).reshape(C_D, 128, N)
        .transpose(1, 0, 2).reshape(128, C_D * N))


def _arrange_xt(Xt, nblk):
    """[D, R] f32 (transposed input) -> bf16 [128, nblk*C_D*512],
    columns ordered (block, chunk, col)."""
    Dn, R = Xt.shape
    return np.ascontiguousarray(
        Xt.astype(ml_dtypes.bfloat16).reshape(C_D, 128, nblk, 512)
        .transpose(1, 2, 0, 3).reshape(128, nblk * C_D * 512))


def _arrange_v(V):
    """[S, VD] f32 -> bf16 [128, 2*KT*512], columns (vd-half, kt, col)."""
    return np.ascontiguousarray(
        V.astype(ml_dtypes.bfloat16).reshape(KT, 128, 2, 512)
        .transpose(1, 2, 0, 3).reshape(128, 2 * KT * 512))


def make_in_maps(query, key, value, Wq, bq, Wk, bk, Wv, bv):
    Wqh = _arrange_w(Wq)
    Wkh = _arrange_w(Wk)
    Wvh = _arrange_w(Wv)
    bqk = np.ascontiguousarray(
        np.concatenate([bq.reshape(8, 128).T, bk.reshape(8, 128).T], axis=1)
        .astype(np.float32))
    kThs = [_arrange_xt(key[b].T, NKB) for b in range(B)]
    v16hs = [_arrange_v(value[b]) for b in range(B)]
    in_maps = []
    for core in range(N_CORES):
        b, h = divmod(core, 2)
        in_maps.append({
            "qTh": _arrange_xt(query[b, h * QS:(h + 1) * QS, :].T, NQB),
            "kTh": kThs[b],
            "v16h": v16hs[b],
            "Wqh": Wqh, "Wkh": Wkh, "Wvh": Wvh,
            "bqk": bqk, "bv": bv,
        })
    return in_maps


# revision 6
# speedup vs baseline: 1.1100x; 1.1100x over previous
"""Trainium2 Bass kernel for batched dense attention (v3).

Reference (per batch b):
    q = query @ Wq + bq ; k = key @ Wk + bk ; v = value @ Wv + bv
    out = softmax(BETA * q k^T) v

Sharding: 8 cores = (batch b, seq half h). Core (b,h) computes out rows
[b, h*1024:(h+1)*1024, :]. K-side work is duplicated across the two cores
of a batch (no collectives).

Design (v2/v3):
  - query/key arrive HOST-TRANSPOSED ([D, rows]) and all matmul inputs are
    host-cast bf16, so the kernel needs zero PE transposes and no dtype
    juggling; PSUM accumulates in f32.
  - Projections stream W/input chunks c-outer into 8 PSUM banks.
  - Phase A: sT[k,q] tiles -> exp on ScalarE -> pT (bf16); row sums of P
    accumulate on PE via lhsT=pT slices, rhs=ones (one PSUM bank).
  - Phase B: out2T[vd',q] = (value^T P)^T using natural-layout value tiles
    as lhsT (contraction over k) -- no transposes.
  - Phase C: out[q,:] = (out2T^T @ Wv) * (1/rowsum) + bv, normalization and
    bias fused in one scalar_tensor_tensor on the PSUM->SBUF copy.
  - v3: DMA batching. Per-dma_start issue cost is ~0.6us of engine time plus
    semaphore latency, so v2's ~60 small prologue DMAs gated the first
    matmul to t=33us. v3 packs biases host-side ([128,16] in one DMA),
    loads W in 1-2 multi-chunk 3D-AP DMAs, qT/kT one DMA per 512-col block,
    value 2 k-tiles per DMA, output 2 col-blocks per DMA. bv-broadcast
    matmuls moved after the projections so PE starts on q-proj immediately.

Note: a v4 that removed the remaining pipeline stalls entirely (host
SBUF-layout prearrangement + g-outer projections + interleaved rowsums)
measured SLOWER on hardware (307.8us vs 276.9us): with the PE stream fully
dense the chip sustains peak power and P0-downclocks the PE from 2.4GHz to
~2.0GHz (HAM stays at K=8/8; every N=512 matmul stretches 378->453ns).
v3's brief stalls keep it under the power clamp, so v3 is the faster
configuration in practice.

Per-core PE budget @2.4GHz: qproj 65k + kproj 131k + scores 131k +
phaseB 131k + phaseC 65k ~= 523k cycles ~= 218us + ~15us rowsum overhead.
"""
import ml_dtypes
import numpy as np

import concourse.bass as bass
import concourse.bacc as bacc
import concourse.tile as tile
from concourse import mybir
from concourse.bass_utils import run_bass_kernel_spmd

B, S, D = 4, 2048, 1024
KD = 1024
VD = 1024
BETA = 1.0 / float(np.sqrt(D))
N_CORES = 8
QS = S // 2          # per-core query rows (1024)

F32 = mybir.dt.float32
BF16 = mybir.dt.bfloat16

C_D = D // 128       # 8 contraction chunks over D
G_KD = KD // 128     # 8 kd chunks
KT = S // 128        # 16 key tiles
QBLK = 512
NQB = QS // QBLK     # 2 q blocks
NQS = QBLK // 128    # 4 q slices per block
NKB = S // 512       # 4 key col-blocks


def build_kernel():
    nc = bacc.Bacc("TRN2", target_bir_lowering=False, debug=False,
                   num_devices=N_CORES)

    qT = nc.dram_tensor("qT", [D, QS], BF16, kind="ExternalInput").ap()
    kT = nc.dram_tensor("kT", [D, S], BF16, kind="ExternalInput").ap()
    v16 = nc.dram_tensor("v16", [S, VD], BF16, kind="ExternalInput").ap()
    Wq = nc.dram_tensor("Wq", [D, KD], BF16, kind="ExternalInput").ap()
    Wk = nc.dram_tensor("Wk", [D, KD], BF16, kind="ExternalInput").ap()
    Wv16 = nc.dram_tensor("Wv16", [VD, VD], BF16, kind="ExternalInput").ap()
    bqk = nc.dram_tensor("bqk", [128, 16], F32, kind="ExternalInput").ap()
    bv = nc.dram_tensor("bv", [VD], F32, kind="ExternalInput").ap()
    out = nc.dram_tensor("out", [QS, VD], F32, kind="ExternalOutput").ap()

    with tile.TileContext(nc) as tc:
        _body(tc, qT, kT, v16, Wq, Wk, Wv16, bqk, bv, out)
    nc.compile()
    return nc


def _chunked(dram_ap, rows0, nchunk, cols):
    """[nchunk*128, cols] DRAM slice as a [128, nchunk, cols] AP."""
    sl = dram_ap[rows0:rows0 + nchunk * 128, 0:cols] if cols else dram_ap
    return sl.rearrange("(c p) x -> p c x", c=nchunk)


def _body(tc, qT, kT, v16, Wq, Wk, Wv16, bqk, bv, out):
    nc = tc.nc
    Exp = mybir.ActivationFunctionType.Exp
    mult = mybir.AluOpType.mult
    add = mybir.AluOpType.add

    # ---- persistent constants ------------------------------------------
    # constf cols: [0:8]=bqT, [8:16]=bkT, [16:1040]=bvb, [1040:1042]=ones,
    # [1042:1058]=rrec (2 qb x 8), row0 [1058:1186]=ones row,
    # row0 [1186:2210]=bv staging
    const_pool = tc.alloc_tile_pool(name="const", bufs=1)
    constf = const_pool.tile([128, 2210], F32, name="constf")
    bqT = constf[:, 0:8]
    bkT = constf[:, 8:16]
    bvb = constf[:, 16:16 + VD]
    ones_f = constf[:, 1040:1042]
    rrec_all = constf[:, 1042:1058]
    onesrow_f = constf[0:1, 1058:1058 + 128]
    bv_stage = constf[0:1, 1186:1186 + VD]
    onesb = const_pool.tile([128, 2], BF16, name="onesb")

    nc.scalar.dma_start(out=constf[:, 0:16], in_=bqk[:, :])
    nc.scalar.dma_start(out=bv_stage, in_=bv[:])
    nc.vector.memset(ones_f, 1.0)
    nc.vector.memset(onesrow_f, 1.0)
    nc.vector.tensor_copy(onesb[:], ones_f)

    # ---- persistent activations ----------------------------------------
    big_pool = tc.alloc_tile_pool(name="big", bufs=1)
    qTr = big_pool.tile([128, G_KD * QS], BF16, name="qTr")       # 16KB/p
    kTr = big_pool.tile([128, G_KD * S], BF16, name="kTr")        # 32KB/p
    Wv_sb = big_pool.tile([128, C_D * VD], BF16, name="Wv_sb")    # 16KB/p
    pT = big_pool.tile([128, KT * QBLK], BF16, name="pT")         # 16KB/p
    o2T = big_pool.tile([128, C_D * QBLK], BF16, name="o2T")      # 8KB/p
    ost_all = big_pool.tile([128, 2 * 1024], F32, name="ost_all")  # 8KB/p
    ostage = [ost_all[:, i * 1024:(i + 1) * 1024] for i in range(2)]

    # Wv whole in one DMA on gpsimd (needed only at phase C)
    nc.gpsimd.dma_start(out=Wv_sb[:].rearrange("p (c x) -> p c x", c=C_D),
                        in_=_chunked(Wv16, 0, C_D, VD))

    # ---- projection-phase transients -----------------------------------
    proj_pool = tc.alloc_tile_pool(name="proj", bufs=1)
    Wq_sb = proj_pool.tile([128, C_D * KD], BF16, name="Wq_sb")   # 16KB/p
    Wk_sb = proj_pool.tile([128, C_D * KD], BF16, name="Wk_sb")   # 16KB/p
    qblk = [proj_pool.tile([128, C_D * 512], BF16, name=f"qb{i}")
            for i in range(NQB)]                                  # 2x8KB/p
    kblk = [proj_pool.tile([128, C_D * 512], BF16, name=f"kb{i}")
            for i in range(NKB)]                                  # 4x8KB/p

    # sync queue: Wq halves interleaved with qT blocks, then kT blocks 2-3
    Wq_v = Wq_sb[:].rearrange("p (c x) -> p c x", c=C_D)
    Wk_v = Wk_sb[:].rearrange("p (c x) -> p c x", c=C_D)
    for h in range(2):
        nc.sync.dma_start(out=Wq_v[:, 4 * h:4 * h + 4, :],
                          in_=_chunked(Wq, h * 512, 4, KD))
        nc.sync.dma_start(
            out=qblk[h][:].rearrange("p (c x) -> p c x", c=C_D),
            in_=qT[:, h * 512:(h + 1) * 512].rearrange(
                "(c p) x -> p c x", c=C_D))
    # scalar queue: biases above, then Wk whole, kT blocks 0-1
    nc.scalar.dma_start(out=Wk_v[:, :, :], in_=_chunked(Wk, 0, C_D, KD))
    for blk in range(NKB):
        eng = nc.scalar if blk < 2 else nc.sync
        eng.dma_start(
            out=kblk[blk][:].rearrange("p (c x) -> p c x", c=C_D),
            in_=kT[:, blk * 512:(blk + 1) * 512].rearrange(
                "(c p) x -> p c x", c=C_D))

    psPro = tc.alloc_tile_pool(name="psPro", bufs=1, space="PSUM")

    # ---- q projection: qTr[kd, q] = (Wq^T qT) + bq ----------------------
    for blk in range(NQB):
        pps = [psPro.tile([128, 512], F32, name=f"qp{blk}_{g}", tag="pp",
                          bufs=8) for g in range(G_KD)]
        for c in range(C_D):
            for g in range(G_KD):
                nc.tensor.matmul(
                    pps[g][:],
                    Wq_sb[:, c * KD + g * 128:c * KD + (g + 1) * 128],
                    qblk[blk][:, c * 512:(c + 1) * 512],
                    start=(c == 0), stop=(c == C_D - 1))
        for g in range(G_KD):
            nc.vector.tensor_scalar(
                out=qTr[:, g * QS + blk * 512:g * QS + (blk + 1) * 512],
                in0=pps[g][:], scalar1=bqT[:, g:g + 1], scalar2=None, op0=add)

    # ---- k projection: kTr[kd, k] = (Wk^T kT) + bk ----------------------
    for blk in range(NKB):
        pps = [psPro.tile([128, 512], F32, name=f"kp{blk}_{g}", tag="pp",
                          bufs=8) for g in range(G_KD)]
        for c in range(C_D):
            for g in range(G_KD):
                nc.tensor.matmul(
                    pps[g][:],
                    Wk_sb[:, c * KD + g * 128:c * KD + (g + 1) * 128],
                    kblk[blk][:, c * 512:(c + 1) * 512],
                    start=(c == 0), stop=(c == C_D - 1))
        for g in range(G_KD):
            nc.vector.tensor_scalar(
                out=kTr[:, g * S + blk * 512:g * S + (blk + 1) * 512],
                in0=pps[g][:], scalar1=bkT[:, g:g + 1], scalar2=None, op0=add)

    # bv broadcast to all partitions via K=1 fp32 matmul (off critical path)
    for n in range(VD // 512):
        bc_ps = psPro.tile([128, 512], F32, name="bc_ps", tag="pp", bufs=8)
        nc.tensor.matmul(bc_ps[:], onesrow_f,
                         bv_stage[:, n * 512:(n + 1) * 512],
                         start=True, stop=True)
        nc.vector.tensor_copy(bvb[:, n * 512:(n + 1) * 512], bc_ps[:])

    psPro.release()
    proj_pool.release()

    # ===== main attention loop ==========================================
    # PSUM: sT(2) + rs(1) + acc(4) = 7 banks.
    psM = tc.alloc_tile_pool(name="psM", bufs=1, space="PSUM")
    rs_ps = psM.tile([128, 2 * NQS], F32, name="rs_ps", tag="rs")

    for qb in range(NQB):
        q0 = qb * QBLK
        # ---- phase A: sT = kTr^T qTr -> exp -> pT ; rowsums on PE ------
        for kt in range(KT):
            sT = psM.tile([128, QBLK], F32, name=f"sT{qb}_{kt}", tag="sT",
                          bufs=2)
            for g in range(G_KD):
                nc.tensor.matmul(
                    sT[:],
                    kTr[:, g * S + kt * 128:g * S + (kt + 1) * 128],
                    qTr[:, g * QS + q0:g * QS + q0 + QBLK],
                    start=(g == 0), stop=(g == G_KD - 1))
            nc.scalar.activation(pT[:, kt * QBLK:(kt + 1) * QBLK], sT[:],
                                 Exp, scale=float(BETA))
            for qs in range(NQS):
                nc.tensor.matmul(
                    rs_ps[:, 2 * qs:2 * qs + 2],
                    pT[:, kt * QBLK + qs * 128:kt * QBLK + (qs + 1) * 128],
                    onesb[:],
                    start=(kt == 0 and qs == 0),
                    stop=(kt == KT - 1 and qs == NQS - 1),
                    skip_group_check=True)
        rrec = rrec_all[:, qb * 2 * NQS:(qb + 1) * 2 * NQS]
        nc.vector.reciprocal(rrec, rs_ps[:])

        # ---- phase B: o2T[vd', q] = (value^T P)^T via lhsT=value tiles --
        for p in range(2):
            accs = [psM.tile([128, QBLK], F32, name=f"o2{qb}_{p}_{u}",
                             tag="acc", bufs=4) for u in range(4)]
            for kt2 in range(KT // 2):
                vt = big_pool.tile([128, 2 * 512], BF16,
                                   name=f"vt{qb}_{p}_{kt2}", tag="vring",
                                   bufs=4)
                nc.gpsimd.dma_start(
                    out=vt[:].rearrange("p (c x) -> p c x", c=2),
                    in_=v16[kt2 * 256:(kt2 + 1) * 256,
                            p * 512:(p + 1) * 512].rearrange(
                        "(c p) x -> p c x", c=2))
                for j in range(2):
                    kt = 2 * kt2 + j
                    for u in range(4):
                        nc.tensor.matmul(
                            accs[u][:],
                            vt[:, j * 512 + u * 128:j * 512 + (u + 1) * 128],
                            pT[:, kt * QBLK:(kt + 1) * QBLK],
                            start=(kt == 0), stop=(kt == KT - 1))
            for u in range(4):
                nc.vector.tensor_copy(
                    o2T[:, (4 * p + u) * QBLK:(4 * p + u + 1) * QBLK],
                    accs[u][:])

        # ---- phase C: out = (o2T^T Wv) * rrec + bv ----------------------
        for qs in range(NQS):
            ost = ostage[qs % 2]
            for col in range(2):
                op = psM.tile([128, 512], F32, name=f"op{qb}_{qs}_{col}",
                              tag="acc", bufs=4)
                for cp in range(C_D):
                    nc.tensor.matmul(
                        op[:],
                        o2T[:, cp * QBLK + qs * 128:cp * QBLK + (qs + 1) * 128],
                        Wv_sb[:, cp * VD + col * 512:cp * VD + (col + 1) * 512],
                        start=(cp == 0), stop=(cp == C_D - 1))
                nc.vector.scalar_tensor_tensor(
                    out=ost[:, col * 512:(col + 1) * 512], in0=op[:],
                    scalar=rrec[:, 2 * qs:2 * qs + 1],
                    in1=bvb[:, col * 512:(col + 1) * 512], op0=mult, op1=add)
            nc.sync.dma_start(
                out=out[q0 + qs * 128:q0 + (qs + 1) * 128, :], in_=ost[:])

    psM.release()
    big_pool.release()
    const_pool.release()


_NC_CACHE = {}


def _get_nc():
    if "nc" not in _NC_CACHE:
        _NC_CACHE["nc"] = build_kernel()
    return _NC_CACHE["nc"]


def kernel(query, key, value, Wq, bq, Wk, bk, Wv, bv):
    query = np.asarray(query, dtype=np.float32)
    key = np.asarray(key, dtype=np.float32)
    value = np.asarray(value, dtype=np.float32)
    Wq = np.asarray(Wq, dtype=np.float32)
    Wk = np.asarray(Wk, dtype=np.float32)
    Wv = np.asarray(Wv, dtype=np.float32)
    bq = np.asarray(bq, dtype=np.float32)
    bk = np.asarray(bk, dtype=np.float32)
    bv = np.ascontiguousarray(np.asarray(bv, dtype=np.float32))

    nc = _get_nc()
    in_maps = make_in_maps(query, key, value, Wq, bq, Wk, bk, Wv, bv)
    res = run_bass_kernel_spmd(nc, in_maps, list(range(N_CORES)))
    outp = np.empty((B, S, VD), dtype=np.float32)
    for core in range(N_CORES):
        b, h = divmod(core, 2)
        outp[b, h * QS:(h + 1) * QS, :] = res.results[core]["out"]
    return outp


def make_in_maps(query, key, value, Wq, bq, Wk, bk, Wv, bv):
    bf16 = ml_dtypes.bfloat16
    Wq16 = Wq.astype(bf16)
    Wk16 = Wk.astype(bf16)
    Wv16 = Wv.astype(bf16)
    bqk = np.ascontiguousarray(
        np.concatenate([bq.reshape(8, 128).T, bk.reshape(8, 128).T], axis=1)
        .astype(np.float32))
    kTs = [np.ascontiguousarray(key[b].T.astype(bf16)) for b in range(B)]
    v16s = [np.ascontiguousarray(value[b].astype(bf16)) for b in range(B)]
    in_maps = []
    for core in range(N_CORES):
        b, h = divmod(core, 2)
        in_maps.append({
            "qT": np.ascontiguousarray(query[b, h * QS:(h + 1) * QS, :].T
                                       .astype(bf16)),
            "kT": kTs[b],
            "v16": v16s[b],
            "Wq": Wq16, "Wk": Wk16, "Wv16": Wv16,
            "bqk": bqk, "bv": bv,
        })
    return in_maps
